# revision 26
# baseline (speedup 1.0000x reference)
"""VQ codebook argmin kernel for Trainium2 (8 NeuronCores, data-parallel).

Problem: latent [131072, 128] f32, coords [2048, 128] f32
         -> argmin_j ||latent_i - coords_j||^2  (int32 [131072])

Math: argmin_j (x2_i + c2_j - 2*cross_ij) = argmax_j (cross_ij - c2_j/2)
so per row we need the argmax of v = latent @ coords.T - h, h = |c|^2/2.

Device algorithm per 128-row tile (rows on partitions):
  1. PE: cross tile [128, 2048] in PSUM (4 matmuls, lhsT = latentT tile
     [128f x 128r], rhs = coordsT [128f x 2048c]).
  2. DVE: one fused custom-DVE pass: m = running-max-scan(cross - h)
     (PSUM+SBUF -> SBUF). The last scan element is the row max v*.
  3. ACT: one pass: out = Sign(v* - m), accum_out = sum = count of
     positions where the running max is still below v* = the index of the
     FIRST position achieving the max = argmax with jnp.argmin tie-break.
Host: shard latent rows 8 ways (pre-transposed per shard), replicate
coords; gather per-core counts and cast to int32.

Measured (axon, per-dispatch minus ~5.0ms dispatch floor) / cost-model
makespan per core:
  fused   (fp32 matmul, default): ~0.40ms   / 453us  - 1/131072 mismatch
                                  (a genuine 1.8e-5 fp64 near-tie)
  fused_f32r (f32r matmul)      : ~0.31ms   / 311us  - 47/131072 mismatches
                                  (f32r is TF32-like; mean rel err 6.7e-4)
fp32 matmuls stream at 4 cycles/column on TRN2 ("2 half-speed matmuls"),
so the fp32 PE floor is ~437us/core; 'fused' sits at that roofline with
the DVE scan (289us) and ACT count (262us) fully overlapped. 'fused_f32r'
is DVE-bound instead; switch DEFAULT_VARIANT to it if ~5e-4 index error
is acceptable.
"""

import numpy as np

import concourse.bass as bass
import concourse.bacc as bacc
import concourse.mybir as mybir
import concourse.tile as tile
import concourse.dve_ops as dve_ops
from concourse.dve_ops import DveOp
from concourse.dve_spec import Spec, Src0, Src1, AluOp, lower, _has_src1, scan
from concourse.dve_uop import DveOpSpec

P = 128          # partitions / rows per tile
D = 128          # feature dim
C = 2048         # n centroids
N_CORES = 8
FULL_ROWS = 131072
SHARD = FULL_ROWS // N_CORES      # 16384
MM_N = 512                        # fp32 moving-operand max

F32 = mybir.dt.float32

# ---------------------------------------------------------------- custom op
_OP_NAME = "SUB_SCANMAX_ANT"


def _register_scanmax_op() -> DveOp:
    """out[p, k] = max over j<=k of (in0[p, j] - in1[p, j]).

    Registered dynamically into dve_ops.OPS (shas computed at import, same
    process does both codegen and table-gen so the registry stays coherent).
    """
    for op in dve_ops.OPS:
        if op.name == _OP_NAME:
            return op
    spec = Spec(
        body=scan(AluOp.MAX, Src0 - Src1),
        reference=lambda in0, in1, s0, s1, imm2: np.maximum.accumulate(
            in0.astype(np.float32) - in1.astype(np.float32), axis=-1
        ),
    )
    row = 1 + len(dve_ops.OPS)
    shas = {
        ver: DveOpSpec(
            name=_OP_NAME, opcode=row, uops=lower(spec, ver=ver),
            rd1_en=_has_src1(spec),
        ).sha(ver)
        for ver in ("v3", "v4")
    }
    op = DveOp(_OP_NAME, spec, subdim=False, uops_sha=shas)
    dve_ops.OPS.append(op)
    dve_ops.CUSTOM_DVE_SPECS[_OP_NAME] = op.spec
    dve_ops._SUB_OPCODE_FOR_NAME[_OP_NAME] = row
    return op


SCANMAX = _register_scanmax_op()


# ---------------------------------------------------------------- kernel IR
def build_nc(n_tiles: int, variant: str = "fused", count_engine: str = "act"):
    """Build the per-core Bass program.

    n_tiles: number of 128-row tiles this core processes.
    variant: 'fused' (custom DVE op) or 'stock' (tensor_sub + tensor_tensor_scan).
    count_engine: 'act' (Sign+accum), 'dve' (STT is_lt + accum) fallback.
    """
    rows = n_tiles * P
    # float32r: same 4-byte storage as fp32, but the PE streams it at
    # 1 cycle/column instead of fp32's 4 (cost model: fp32 = "2 half-speed
    # matmuls"). Used for the matmul operands only; PSUM accumulation stays
    # fp32. 'f32r' in the variant name opts in.
    mm_dt = mybir.dt.float32r if variant.endswith("f32r") else F32
    nc = bacc.Bacc("TRN2", target_bir_lowering=False, debug=False)
    latT = nc.dram_tensor("latT", [D, rows], mm_dt, kind="ExternalInput").ap()
    coordsT = nc.dram_tensor("coordsT", [D, C], mm_dt, kind="ExternalInput").ap()
    hb = nc.dram_tensor("hb", [P, C], F32, kind="ExternalInput").ap()
    variant = variant.replace("_f32r", "")
    if variant == "rank1":
        ones1 = nc.dram_tensor("ones1", [1, P], F32, kind="ExternalInput").ap()
        hneg1 = nc.dram_tensor("hneg1", [1, C], F32, kind="ExternalInput").ap()
    out_dt = mybir.dt.uint32 if variant == "max8" else F32
    out = nc.dram_tensor("out", [P, n_tiles], out_dt, kind="ExternalOutput").ap()

    # Load the latent shard in independent chunks so compute starts after
    # chunk 0 instead of after the whole 8MB (single-queue SWDGE would
    # serialize ~300us of load ahead of the first matmul).
    n_chunks = max(1, min(16, n_tiles))
    while n_tiles % n_chunks:
        n_chunks -= 1
    tpc = n_tiles // n_chunks

    with tile.TileContext(nc) as tc:
        with (
            tc.tile_pool(name="const", bufs=1) as cpool,
            tc.tile_pool(name="lat", bufs=n_chunks) as lpool,
            tc.tile_pool(name="scan", bufs=3) as mpool,
            tc.tile_pool(name="scr", bufs=2) as spool,
            tc.tile_pool(name="oacc", bufs=1) as opool,
            tc.tile_pool(name="ps", bufs=2, space="PSUM") as pspool,
        ):
            ct = cpool.tile([P, C], mm_dt)
            nc.gpsimd.dma_start(out=ct[:], in_=coordsT)
            ht = cpool.tile([P, C], F32)
            nc.gpsimd.dma_start(out=ht[:], in_=hb)
            oacc = opool.tile([P, n_tiles], out_dt)

            if variant == "rank1":
                onest = cpool.tile([1, P], F32)
                nc.gpsimd.dma_start(out=onest[:], in_=ones1)
                hnt = cpool.tile([1, C], F32)
                nc.gpsimd.dma_start(out=hnt[:], in_=hneg1)

            # Self-loading fp32 matmuls (LDW struct) accept only ONE sync
            # wait. Re-write every matmul-read tile in place on ACT so all
            # matmul input deps collapse onto the single ACT proc tick.
            nc.scalar.copy(ct[:], ct[:])
            if variant == "rank1":
                nc.scalar.copy(onest[:], onest[:])
                nc.scalar.copy(hnt[:], hnt[:])

            # whole latent shard stays SBUF-resident (64KB/partition total),
            # one tile per chunk so Tile tracks chunk deps independently.
            lat_chunks = []
            for ci in range(n_chunks):
                lc = lpool.tile([P, tpc * P], mm_dt, tag="latc")
                nc.sync.dma_start(
                    out=lc[:], in_=latT[:, ci * tpc * P:(ci + 1) * tpc * P])
                nc.scalar.copy(lc[:], lc[:])
                lat_chunks.append(lc)

            for t in range(n_tiles):
                lat = lat_chunks[t // tpc][:, (t % tpc) * P:(t % tpc + 1) * P]
                ps = pspool.tile([P, C], F32)
                for k in range(C // MM_N):
                    sl = slice(k * MM_N, (k + 1) * MM_N)
                    nc.tensor.matmul(
                        ps[:, sl], lhsT=lat, rhs=ct[:, sl],
                        start=True, stop=(variant != "rank1"),
                    )
                    if variant == "rank1":
                        # accumulate ones^T @ (-h) so PSUM holds cross - h
                        nc.tensor.matmul(
                            ps[:, sl], lhsT=onest[:], rhs=hnt[:, sl],
                            start=False, stop=True,
                        )

                if variant == "max8":
                    v = mpool.tile([P, C], F32, tag="vtmp")
                    nc.vector.tensor_sub(v[:], ps[:], ht[:])
                    mx = spool.tile([P, 8], F32, tag="mx8")
                    nc.vector.max(out=mx[:], in_=v[:])
                    ix = spool.tile([P, 8], mybir.dt.uint32, tag="ix8")
                    nc.vector.max_index(ix[:], mx[:], v[:])
                    nc.vector.tensor_copy(oacc[:, t:t + 1], ix[:, 0:1])
                    continue

                m = mpool.tile([P, C], F32)
                if variant == "fused":
                    nc.vector._custom_dve(SCANMAX, out=m[:], in0=ps[:], in1=ht[:])
                elif variant == "rank1":
                    nc.vector.tensor_tensor_scan(
                        out=m[:], data0=ps[:], data1=ht[:],
                        initial=-3.0e38,
                        op0=mybir.AluOpType.max, op1=mybir.AluOpType.bypass,
                    )
                else:
                    v = mpool.tile([P, C], F32, tag="vtmp")
                    nc.vector.tensor_sub(v[:], ps[:], ht[:])
                    nc.vector.tensor_tensor_scan(
                        out=m[:], data0=v[:], data1=v[:],
                        initial=-3.0e38,
                        op0=mybir.AluOpType.max, op1=mybir.AluOpType.bypass,
                    )

                vstar = m[:, C - 1:C]
                if count_engine == "act":
                    sgn = spool.tile([P, C], F32)
                    nc.scalar.activation(
                        out=sgn[:], in_=m[:],
                        func=mybir.ActivationFunctionType.Sign,
                        bias=vstar, scale=-1.0,
                        accum_out=oacc[:, t:t + 1],
                    )
                else:  # 'dve' fallback: exact ALU compare + accum on DVE
                    lt = spool.tile([P, C], F32)
                    nc.vector.scalar_tensor_tensor(
                        out=lt[:], in0=m[:], scalar=vstar, in1=m[:],
                        op0=mybir.AluOpType.is_lt,
                        op1=mybir.AluOpType.bypass,
                        accum_out=oacc[:, t:t + 1],
                    )

            nc.gpsimd.dma_start(out=out, in_=oacc[:])

    _strip_pe_self_waits(nc)
    # Bacc defers reg-alloc / wait-splitting to its compile pipeline, which
    # runs in finalize(); the bass2jax/axon exec path does not call it.
    nc.finalize()
    return nc


def _strip_pe_self_waits(nc):
    """Self-loading fp32 matmuls lower to an LDW struct that accepts only ONE
    sync wait. Tile emits a redundant same-engine (PE-sem) wait for PSUM-slot
    WAW reuse on top of the cross-engine reader-release wait; MM execution is
    strict-FIFO on PE (and LDW never touches PSUM/SBUF-writes), so the
    same-engine wait is timing-irrelevant. Drop PE-updated sems from matmul
    waits when more than one wait is present."""
    pe_sems = set()
    for blk in nc.m.functions[0].blocks:
        for i in blk.instructions:
            if getattr(i, "engine", None) == mybir.EngineType.PE and i.sync_info:
                for u in i.sync_info.on_update:
                    pe_sems.add(u.ant_name)
    for blk in nc.m.functions[0].blocks:
        for i in blk.instructions:
            if type(i).__name__ not in ("InstMatmult", "InstLdweights"):
                continue
            si = i.sync_info
            if not si or len(si.on_wait) <= 1:
                continue
            kept = [w for w in si.on_wait if w.ant_name not in pe_sems]
            if len(kept) != len(si.on_wait):
                assert kept, f"{i.name}: all waits were PE-self waits"
                si.on_wait = kept
                i.sync_info = si


# ---------------------------------------------------------------- host side
def _prep_core_inputs(latent: np.ndarray, coords: np.ndarray,
                      variant: str = "fused"):
    coords = np.asarray(coords, dtype=np.float32)
    latent = np.asarray(latent, dtype=np.float32)
    c2 = np.sum(coords * coords, axis=1, dtype=np.float32)
    h = (0.5 * c2).astype(np.float32)
    coordsT = np.ascontiguousarray(coords.T)                    # [128, 2048]
    hb = np.ascontiguousarray(np.broadcast_to(h[None, :], (P, C)))
    in_maps = []
    for i in range(N_CORES):
        shard = latent[i * SHARD:(i + 1) * SHARD]
        m = {
            "latT": np.ascontiguousarray(shard.T),              # [128, 16384]
            "coordsT": coordsT,
            "hb": hb,
        }
        if variant == "rank1":
            m["ones1"] = np.ones((1, P), np.float32)
            m["hneg1"] = np.ascontiguousarray(-h[None, :])
        in_maps.append(m)
    return in_maps


_NC_CACHE: dict = {}


def _get_nc(variant: str, count_engine: str):
    key = (variant, count_engine, SHARD // P)
    if key not in _NC_CACHE:
        _NC_CACHE[key] = build_nc(SHARD // P, variant, count_engine)
    return _NC_CACHE[key]


def run_on_cores(latent, coords, variant="fused", count_engine="act",
                 trace=False):
    from concourse.bass_utils import run_bass_kernel_spmd

    nc = _get_nc(variant, count_engine)
    in_maps = _prep_core_inputs(latent, coords, variant)
    res = run_bass_kernel_spmd(nc, in_maps, core_ids=list(range(N_CORES)),
                               trace=trace)
    shards = []
    for i in range(N_CORES):
        o = res.results[i]["out"]                    # [128, n_tiles]
        o = np.rint(o) if o.dtype == np.float32 else o
        shards.append(o.astype(np.int32).T.reshape(-1))
    return np.concatenate(shards), res


DEFAULT_VARIANT = "fused"      # see module docstring; "fused_f32r" is faster
                               # but trades index accuracy (TF32-like matmul)


def kernel(latent: np.ndarray, coords: np.ndarray) -> np.ndarray:
    idx, _ = run_on_cores(latent, coords, variant=DEFAULT_VARIANT)
    return idx


# revision 35
# speedup vs baseline: 1.5050x; 1.5050x over previous
"""VQ codebook argmin kernel for Trainium2 (8 NeuronCores, data-parallel).

Problem: latent [131072, 128] f32, coords [2048, 128] f32
         -> argmin_j ||latent_i - coords_j||^2  (int32 [131072])

Math: argmin_j (x2_i + c2_j - 2*cross_ij) = argmax_j (cross_ij - c2_j/2)
so per row we need the argmax of v = latent @ coords.T - h, h = |c|^2/2.

Device algorithm per 128-row tile (rows on partitions):
  1. PE: cross tile [128, 2048] in PSUM (4 matmuls, lhsT = latentT tile
     [128f x 128r], rhs = coordsT [128f x 2048c]).
  2. DVE: one fused custom-DVE pass: m = running-max-scan(cross - h)
     (PSUM+SBUF -> SBUF). The last scan element is the row max v*.
  3. ACT: one pass: out = Sign(v* - m), accum_out = sum = count of
     positions where the running max is still below v* = the index of the
     FIRST position achieving the max = argmax with jnp.argmin tie-break.
Host: shard latent rows 8 ways (pre-transposed per shard), replicate
coords; gather per-core counts and cast to int32.

Measured (axon, per-dispatch minus ~5.0ms dispatch floor) / cost-model
makespan per core:
  fused   (fp32 matmul, default): ~0.40ms   / 453us  - 1/131072 mismatch
                                  (a genuine 1.8e-5 fp64 near-tie)
  fused_f32r (f32r matmul)      : ~0.31ms   / 311us  - 47/131072 mismatches
                                  (f32r is TF32-like; mean rel err 6.7e-4)
fp32 matmuls stream at 4 cycles/column on TRN2 ("2 half-speed matmuls"),
so the fp32 PE floor is ~437us/core; 'fused' sits at that roofline with
the DVE scan (289us) and ACT count (262us) fully overlapped. 'fused_f32r'
is DVE-bound instead; switch DEFAULT_VARIANT to it if ~5e-4 index error
is acceptable.
"""

import numpy as np

import concourse.bass as bass
import concourse.bacc as bacc
import concourse.mybir as mybir
import concourse.tile as tile
import concourse.dve_ops as dve_ops
from concourse.dve_ops import DveOp
from concourse.dve_spec import Spec, Src0, Src1, AluOp, lower, _has_src1, scan
from concourse.dve_uop import DveOpSpec

P = 128          # partitions / rows per tile
D = 128          # feature dim
C = 2048         # n centroids
N_CORES = 8
FULL_ROWS = 131072
SHARD = FULL_ROWS // N_CORES      # 16384
MM_N = 512                        # fp32 moving-operand max

F32 = mybir.dt.float32

# ---------------------------------------------------------------- custom op
_OP_NAME = "SUB_SCANMAX_ANT"


def _register_scanmax_op() -> DveOp:
    """out[p, k] = max over j<=k of (in0[p, j] - in1[p, j]).

    Registered dynamically into dve_ops.OPS (shas computed at import, same
    process does both codegen and table-gen so the registry stays coherent).
    """
    for op in dve_ops.OPS:
        if op.name == _OP_NAME:
            return op
    spec = Spec(
        body=scan(AluOp.MAX, Src0 - Src1),
        reference=lambda in0, in1, s0, s1, imm2: np.maximum.accumulate(
            in0.astype(np.float32) - in1.astype(np.float32), axis=-1
        ),
    )
    row = 1 + len(dve_ops.OPS)
    shas = {
        ver: DveOpSpec(
            name=_OP_NAME, opcode=row, uops=lower(spec, ver=ver),
            rd1_en=_has_src1(spec),
        ).sha(ver)
        for ver in ("v3", "v4")
    }
    op = DveOp(_OP_NAME, spec, subdim=False, uops_sha=shas)
    dve_ops.OPS.append(op)
    dve_ops.CUSTOM_DVE_SPECS[_OP_NAME] = op.spec
    dve_ops._SUB_OPCODE_FOR_NAME[_OP_NAME] = row
    return op


SCANMAX = _register_scanmax_op()


# ---------------------------------------------------------------- kernel IR
def build_nc(n_tiles: int, variant: str = "fused", count_engine: str = "act",
             repeats: int = 1):
    """Build the per-core Bass program.

    n_tiles: number of 128-row tiles this core processes.
    variant: 'fused' (custom DVE op) or 'stock' (tensor_sub + tensor_tensor_scan).
    count_engine: 'act' (Sign+accum), 'dve' (STT is_lt + accum) fallback.
    """
    rows = n_tiles * P
    # float32r: same 4-byte storage as fp32, but the PE streams it at
    # 1 cycle/column instead of fp32's 4 (cost model: fp32 = "2 half-speed
    # matmuls"). Used for the matmul operands only; PSUM accumulation stays
    # fp32. 'f32r' in the variant name opts in.
    mm_dt = (mybir.dt.float32r
             if (variant.endswith("f32r") or variant == "limb") else F32)
    nc = bacc.Bacc("TRN2", target_bir_lowering=False, debug=False)
    if variant != "limb":
        latT = nc.dram_tensor("latT", [D, rows], mm_dt,
                              kind="ExternalInput").ap()
    coordsT = nc.dram_tensor("coordsT", [D, C], mm_dt, kind="ExternalInput").ap()
    hb = nc.dram_tensor("hb", [P, C], F32, kind="ExternalInput").ap()
    variant = variant.replace("_f32r", "")
    if variant == "limb":
        # cross = xh.ch + xh.cl + xl.c, xh=bf16(x) (f32r-exact), xl=x-xh.
        # All four product terms captured; only f32r internal roundings
        # (~2^-19 of |x||c|) remain -> fp32-level accuracy at 3 MMs/bank
        # of 1 cyc/col instead of fp32's 4 cyc/col.
        f32r = mybir.dt.float32r
        latHiT = nc.dram_tensor("latHiT", [D, rows], f32r,
                                kind="ExternalInput").ap()
        latLoT = nc.dram_tensor("latLoT", [D, rows], f32r,
                                kind="ExternalInput").ap()
        ctHiT = nc.dram_tensor("ctHiT", [D, C], f32r,
                               kind="ExternalInput").ap()
        ctLoT = nc.dram_tensor("ctLoT", [D, C], f32r,
                               kind="ExternalInput").ap()
    if variant == "rank1":
        ones1 = nc.dram_tensor("ones1", [1, P], F32, kind="ExternalInput").ap()
        hneg1 = nc.dram_tensor("hneg1", [1, C], F32, kind="ExternalInput").ap()
    out_dt = mybir.dt.uint32 if variant == "max8" else F32
    out = nc.dram_tensor("out", [P, n_tiles], out_dt, kind="ExternalOutput").ap()

    # Load the latent shard in independent chunks so compute starts after
    # chunk 0 instead of after the whole 8MB (single-queue SWDGE would
    # serialize ~300us of load ahead of the first matmul).
    n_chunks = max(1, min(16, n_tiles))
    while n_tiles % n_chunks:
        n_chunks -= 1
    tpc = n_tiles // n_chunks

    lat_streams = 2 if variant == "limb" else 1
    with tile.TileContext(nc) as tc:
        with (
            tc.tile_pool(name="const", bufs=1) as cpool,
            tc.tile_pool(name="lat", bufs=n_chunks * lat_streams) as lpool,
            tc.tile_pool(name="scan", bufs=2 if variant == "limb" else 3) as mpool,
            tc.tile_pool(name="scr", bufs=1 if variant == "limb" else 2) as spool,
            tc.tile_pool(name="oacc", bufs=1) as opool,
            tc.tile_pool(name="ps", bufs=2, space="PSUM") as pspool,
        ):
            ct = cpool.tile([P, C], mm_dt)
            nc.gpsimd.dma_start(out=ct[:], in_=coordsT)
            ht = cpool.tile([P, C], F32)
            nc.gpsimd.dma_start(out=ht[:], in_=hb)
            oacc = opool.tile([P, n_tiles], out_dt)

            if variant == "rank1":
                onest = cpool.tile([1, P], F32)
                nc.gpsimd.dma_start(out=onest[:], in_=ones1)
                hnt = cpool.tile([1, C], F32)
                nc.gpsimd.dma_start(out=hnt[:], in_=hneg1)

            # Self-loading fp32 matmuls (LDW struct) accept only ONE sync
            # wait. Re-write every matmul-read tile in place on ACT so all
            # matmul input deps collapse onto the single ACT proc tick.
            nc.scalar.copy(ct[:], ct[:])
            if variant == "rank1":
                nc.scalar.copy(onest[:], onest[:])
                nc.scalar.copy(hnt[:], hnt[:])

            # whole latent shard stays SBUF-resident (64KB/partition per
            # stream), one tile per chunk so Tile tracks deps independently.
            lat_chunks, lo_chunks = [], []
            if variant == "limb":
                cth = cpool.tile([P, C], mybir.dt.float32r)
                nc.gpsimd.dma_start(out=cth[:], in_=ctHiT)
                ctl = cpool.tile([P, C], mybir.dt.float32r)
                nc.gpsimd.dma_start(out=ctl[:], in_=ctLoT)
                nc.scalar.copy(cth[:], cth[:])
                nc.scalar.copy(ctl[:], ctl[:])
                for ci in range(n_chunks):
                    sl = slice(ci * tpc * P, (ci + 1) * tpc * P)
                    lh = lpool.tile([P, tpc * P], mybir.dt.float32r, tag="latc")
                    nc.sync.dma_start(out=lh[:], in_=latHiT[:, sl])
                    nc.scalar.copy(lh[:], lh[:])
                    lat_chunks.append(lh)
                    ll = lpool.tile([P, tpc * P], mybir.dt.float32r, tag="latc")
                    nc.sync.dma_start(out=ll[:], in_=latLoT[:, sl])
                    nc.scalar.copy(ll[:], ll[:])
                    lo_chunks.append(ll)
            else:
                for ci in range(n_chunks):
                    lc = lpool.tile([P, tpc * P], mm_dt, tag="latc")
                    nc.sync.dma_start(
                        out=lc[:], in_=latT[:, ci * tpc * P:(ci + 1) * tpc * P])
                    nc.scalar.copy(lc[:], lc[:])
                    lat_chunks.append(lc)

            # repeats>1 re-runs the whole tile pipeline in one NEFF (same
            # outputs overwritten) — used only to measure steady-state
            # kernel time as a slope, independent of dispatch overhead.
            for t in [t for _ in range(repeats) for t in range(n_tiles)]:
                tsl = slice((t % tpc) * P, (t % tpc + 1) * P)
                lat = lat_chunks[t // tpc][:, tsl]
                ps = pspool.tile([P, C], F32)
                for k in range(C // MM_N):
                    sl = slice(k * MM_N, (k + 1) * MM_N)
                    if variant == "limb":
                        lo = lo_chunks[t // tpc][:, tsl]
                        nc.tensor.matmul(ps[:, sl], lhsT=lat, rhs=cth[:, sl],
                                         start=True, stop=False)
                        nc.tensor.matmul(ps[:, sl], lhsT=lat, rhs=ctl[:, sl],
                                         start=False, stop=False)
                        nc.tensor.matmul(ps[:, sl], lhsT=lo, rhs=ct[:, sl],
                                         start=False, stop=True)
                        continue
                    nc.tensor.matmul(
                        ps[:, sl], lhsT=lat, rhs=ct[:, sl],
                        start=True, stop=(variant != "rank1"),
                    )
                    if variant == "rank1":
                        # accumulate ones^T @ (-h) so PSUM holds cross - h
                        nc.tensor.matmul(
                            ps[:, sl], lhsT=onest[:], rhs=hnt[:, sl],
                            start=False, stop=True,
                        )

                if variant == "max8":
                    v = mpool.tile([P, C], F32, tag="vtmp")
                    nc.vector.tensor_sub(v[:], ps[:], ht[:])
                    mx = spool.tile([P, 8], F32, tag="mx8")
                    nc.vector.max(out=mx[:], in_=v[:])
                    ix = spool.tile([P, 8], mybir.dt.uint32, tag="ix8")
                    nc.vector.max_index(ix[:], mx[:], v[:])
                    nc.vector.tensor_copy(oacc[:, t:t + 1], ix[:, 0:1])
                    continue

                m = mpool.tile([P, C], F32)
                if variant in ("fused", "limb"):
                    nc.vector._custom_dve(SCANMAX, out=m[:], in0=ps[:], in1=ht[:])
                elif variant == "rank1":
                    nc.vector.tensor_tensor_scan(
                        out=m[:], data0=ps[:], data1=ht[:],
                        initial=-3.0e38,
                        op0=mybir.AluOpType.max, op1=mybir.AluOpType.bypass,
                    )
                else:
                    v = mpool.tile([P, C], F32, tag="vtmp")
                    nc.vector.tensor_sub(v[:], ps[:], ht[:])
                    nc.vector.tensor_tensor_scan(
                        out=m[:], data0=v[:], data1=v[:],
                        initial=-3.0e38,
                        op0=mybir.AluOpType.max, op1=mybir.AluOpType.bypass,
                    )

                vstar = m[:, C - 1:C]
                if count_engine == "act":
                    sgn = spool.tile([P, C], F32)
                    nc.scalar.activation(
                        out=sgn[:], in_=m[:],
                        func=mybir.ActivationFunctionType.Sign,
                        bias=vstar, scale=-1.0,
                        accum_out=oacc[:, t:t + 1],
                    )
                else:  # 'dve' fallback: exact ALU compare + accum on DVE
                    lt = spool.tile([P, C], F32)
                    nc.vector.scalar_tensor_tensor(
                        out=lt[:], in0=m[:], scalar=vstar, in1=m[:],
                        op0=mybir.AluOpType.is_lt,
                        op1=mybir.AluOpType.bypass,
                        accum_out=oacc[:, t:t + 1],
                    )

            nc.gpsimd.dma_start(out=out, in_=oacc[:])

    _strip_pe_self_waits(nc)
    # Bacc defers reg-alloc / wait-splitting to its compile pipeline, which
    # runs in finalize(); the bass2jax/axon exec path does not call it.
    nc.finalize()
    return nc


def _strip_pe_self_waits(nc):
    """Self-loading fp32 matmuls lower to an LDW struct that accepts only ONE
    sync wait. Tile emits a redundant same-engine (PE-sem) wait for PSUM-slot
    WAW reuse on top of the cross-engine reader-release wait; MM execution is
    strict-FIFO on PE (and LDW never touches PSUM/SBUF-writes), so the
    same-engine wait is timing-irrelevant. Drop PE-updated sems from matmul
    waits when more than one wait is present."""
    pe_sems = set()
    for blk in nc.m.functions[0].blocks:
        for i in blk.instructions:
            if getattr(i, "engine", None) == mybir.EngineType.PE and i.sync_info:
                for u in i.sync_info.on_update:
                    pe_sems.add(u.ant_name)
    for blk in nc.m.functions[0].blocks:
        for i in blk.instructions:
            if type(i).__name__ not in ("InstMatmult", "InstLdweights"):
                continue
            si = i.sync_info
            if not si or len(si.on_wait) <= 1:
                continue
            kept = [w for w in si.on_wait if w.ant_name not in pe_sems]
            if len(kept) != len(si.on_wait):
                assert kept, f"{i.name}: all waits were PE-self waits"
                si.on_wait = kept
                i.sync_info = si


# ---------------------------------------------------------------- host side
def _prep_core_inputs(latent: np.ndarray, coords: np.ndarray,
                      variant: str = "fused"):
    coords = np.asarray(coords, dtype=np.float32)
    latent = np.asarray(latent, dtype=np.float32)
    c2 = np.sum(coords * coords, axis=1, dtype=np.float32)
    h = (0.5 * c2).astype(np.float32)
    coordsT = np.ascontiguousarray(coords.T)                    # [128, 2048]
    hb = np.ascontiguousarray(np.broadcast_to(h[None, :], (P, C)))
    limb = variant.replace("_f32r", "") == "limb"
    if limb:
        import ml_dtypes
        c_hi = coords.astype(ml_dtypes.bfloat16).astype(np.float32)
        c_lo = coords - c_hi
        l_hi = latent.astype(ml_dtypes.bfloat16).astype(np.float32)
        l_lo = latent - l_hi
    in_maps = []
    for i in range(N_CORES):
        sl = slice(i * SHARD, (i + 1) * SHARD)
        m = {"coordsT": coordsT, "hb": hb}
        if limb:
            m["latHiT"] = np.ascontiguousarray(l_hi[sl].T)
            m["latLoT"] = np.ascontiguousarray(l_lo[sl].T)
            m["ctHiT"] = np.ascontiguousarray(c_hi.T)
            m["ctLoT"] = np.ascontiguousarray(c_lo.T)
        else:
            m["latT"] = np.ascontiguousarray(latent[sl].T)      # [128, 16384]
        if variant == "rank1":
            m["ones1"] = np.ones((1, P), np.float32)
            m["hneg1"] = np.ascontiguousarray(-h[None, :])
        in_maps.append(m)
    return in_maps


_NC_CACHE: dict = {}


def _get_nc(variant: str, count_engine: str):
    key = (variant, count_engine, SHARD // P)
    if key not in _NC_CACHE:
        _NC_CACHE[key] = build_nc(SHARD // P, variant, count_engine)
    return _NC_CACHE[key]


def run_on_cores(latent, coords, variant="fused", count_engine="act",
                 trace=False):
    from concourse.bass_utils import run_bass_kernel_spmd

    nc = _get_nc(variant, count_engine)
    in_maps = _prep_core_inputs(latent, coords, variant)
    res = run_bass_kernel_spmd(nc, in_maps, core_ids=list(range(N_CORES)),
                               trace=trace)
    shards = []
    for i in range(N_CORES):
        o = res.results[i]["out"]                    # [128, n_tiles]
        o = np.rint(o) if o.dtype == np.float32 else o
        shards.append(o.astype(np.int32).T.reshape(-1))
    return np.concatenate(shards), res


DEFAULT_VARIANT = "fused"      # see module docstring; "fused_f32r" is faster
                               # but trades index accuracy (TF32-like matmul)


def kernel(latent: np.ndarray, coords: np.ndarray) -> np.ndarray:
    idx, _ = run_on_cores(latent, coords, variant=DEFAULT_VARIANT)
    return idx


# revision 42
# speedup vs baseline: 1.5248x; 1.0132x over previous
"""VQ codebook argmin kernel for Trainium2 (8 NeuronCores, data-parallel).

Problem: latent [131072, 128] f32, coords [2048, 128] f32
         -> argmin_j ||latent_i - coords_j||^2  (int32 [131072])

Math: argmin_j (x2_i + c2_j - 2*cross_ij) = argmax_j (cross_ij - c2_j/2)
so per row we need the argmax of v = latent @ coords.T - h, h = |c|^2/2.

Device algorithm per 128-row tile (rows on partitions):
  1. PE: cross tile [128, 2048] in PSUM (4 matmuls, lhsT = latentT tile
     [128f x 128r], rhs = coordsT [128f x 2048c]).
  2. DVE: one fused custom-DVE pass: m = running-max-scan(cross - h)
     (PSUM+SBUF -> SBUF). The last scan element is the row max v*.
  3. ACT: one pass: out = Sign(v* - m), accum_out = sum = count of
     positions where the running max is still below v* = the index of the
     FIRST position achieving the max = argmax with jnp.argmin tie-break.
Host: shard latent rows 8 ways (pre-transposed per shard), replicate
coords; gather per-core counts and cast to int32.

Variants (cost-model makespan per core / HW-measured index accuracy):
  limb (DEFAULT): f32r limb-split matmul, cross = xh.ch + xh.cl + xl.c
                  with xh = bf16(x) upcast (exact in f32r's ~11-bit
                  internal precision), xl = x - xh exact in fp32. 3 f32r
                  MMs per PSUM bank at 1 cyc/col vs fp32's 4 cyc/col.
                  369us / 2 per 131072 mismatches (rel err 5.9e-06, both
                  genuine fp64 near-ties) -> fp32-class accuracy.
  fused:          plain fp32 matmul. 453us / 1 per 131072 (1.8e-5 fp64
                  near-tie). At the fp32 PE roofline (fp32 streams at
                  4 cyc/col = "2 half-speed matmuls").
  fused_f32r:     raw f32r matmul. 311us / 47 per 131072 (TF32-like,
                  rel err 6.7e-4). DVE-bound.
Engine busy (limb): PE 328us (bottleneck), DVE 289us (one fused scan
pass/tile), ACT ~314us (Sign count + chunk priming) - fully overlapped;
makespan = PE busy + ~42us startup ramp/pipeline fill.
'limb4' (half-tile PSUM buffers + chained scans via SUB_SCANMAX_INIT_ANT)
was sim-validated but models only 2.2us faster - the slack is ramp, not
PSUM stalls - so it is not the default.
"""

import numpy as np

import concourse.bass as bass
import concourse.bacc as bacc
import concourse.mybir as mybir
import concourse.tile as tile
import concourse.dve_ops as dve_ops
from concourse.dve_ops import DveOp
from concourse.dve_spec import Spec, Src0, Src1, AluOp, lower, _has_src1, scan
from concourse.dve_uop import DveOpSpec

P = 128          # partitions / rows per tile
D = 128          # feature dim
C = 2048         # n centroids
N_CORES = 8
FULL_ROWS = 131072
SHARD = FULL_ROWS // N_CORES      # 16384
MM_N = 512                        # fp32 moving-operand max

F32 = mybir.dt.float32

# ---------------------------------------------------------------- custom op
_OP_NAME = "SUB_SCANMAX_ANT"


def _register_scanmax_op() -> DveOp:
    """out[p, k] = max over j<=k of (in0[p, j] - in1[p, j]).

    Registered dynamically into dve_ops.OPS (shas computed at import, same
    process does both codegen and table-gen so the registry stays coherent).
    """
    for op in dve_ops.OPS:
        if op.name == _OP_NAME:
            return op
    spec = Spec(
        body=scan(AluOp.MAX, Src0 - Src1),
        reference=lambda in0, in1, s0, s1, imm2: np.maximum.accumulate(
            in0.astype(np.float32) - in1.astype(np.float32), axis=-1
        ),
    )
    row = 1 + len(dve_ops.OPS)
    shas = {
        ver: DveOpSpec(
            name=_OP_NAME, opcode=row, uops=lower(spec, ver=ver),
            rd1_en=_has_src1(spec),
        ).sha(ver)
        for ver in ("v3", "v4")
    }
    op = DveOp(_OP_NAME, spec, subdim=False, uops_sha=shas)
    dve_ops.OPS.append(op)
    dve_ops.CUSTOM_DVE_SPECS[_OP_NAME] = op.spec
    dve_ops._SUB_OPCODE_FOR_NAME[_OP_NAME] = row
    return op


SCANMAX = _register_scanmax_op()


def _register_scanmax_init_op() -> DveOp:
    """Like SCANMAX but the running max seeds from s0 (per-partition AP) so
    two half-tile scans can chain across PSUM buffers."""
    name = "SUB_SCANMAX_INIT_ANT"
    for op in dve_ops.OPS:
        if op.name == name:
            return op
    from concourse.dve_spec import C0
    spec = Spec(
        body=scan(AluOp.MAX, Src0 - Src1, init=C0),
        reference=lambda in0, in1, s0, s1, imm2: np.maximum.accumulate(
            np.concatenate(
                [np.broadcast_to(np.asarray(s0, np.float32).reshape(-1, 1),
                                 (in0.shape[0], 1)),
                 in0.astype(np.float32) - in1.astype(np.float32)], axis=-1),
            axis=-1)[:, 1:],
    )
    row = 1 + len(dve_ops.OPS)
    shas = {
        ver: DveOpSpec(
            name=name, opcode=row, uops=lower(spec, ver=ver),
            rd1_en=_has_src1(spec),
        ).sha(ver)
        for ver in ("v3", "v4")
    }
    op = DveOp(name, spec, subdim=False, uops_sha=shas)
    dve_ops.OPS.append(op)
    dve_ops.CUSTOM_DVE_SPECS[name] = op.spec
    dve_ops._SUB_OPCODE_FOR_NAME[name] = row
    return op


SCANMAX_INIT = _register_scanmax_init_op()


# ---------------------------------------------------------------- kernel IR
def build_nc(n_tiles: int, variant: str = "fused", count_engine: str = "act",
             repeats: int = 1):
    """Build the per-core Bass program.

    n_tiles: number of 128-row tiles this core processes.
    variant: 'fused' (custom DVE op) or 'stock' (tensor_sub + tensor_tensor_scan).
    count_engine: 'act' (Sign+accum), 'dve' (STT is_lt + accum) fallback.
    """
    rows = n_tiles * P
    # float32r: same 4-byte storage as fp32, but the PE streams it at
    # 1 cycle/column instead of fp32's 4 (cost model: fp32 = "2 half-speed
    # matmuls"). Used for the matmul operands only; PSUM accumulation stays
    # fp32. 'f32r' in the variant name opts in.
    mm_dt = (mybir.dt.float32r
             if (variant.endswith("f32r") or variant.startswith("limb")) else F32)
    nc = bacc.Bacc("TRN2", target_bir_lowering=False, debug=False)
    if not variant.startswith("limb"):
        latT = nc.dram_tensor("latT", [D, rows], mm_dt,
                              kind="ExternalInput").ap()
    coordsT = nc.dram_tensor("coordsT", [D, C], mm_dt, kind="ExternalInput").ap()
    hb = nc.dram_tensor("hb", [P, C], F32, kind="ExternalInput").ap()
    variant = variant.replace("_f32r", "")
    if variant.startswith("limb"):
        # cross = xh.ch + xh.cl + xl.c, xh=bf16(x) (f32r-exact), xl=x-xh.
        # All four product terms captured; only f32r internal roundings
        # (~2^-19 of |x||c|) remain -> fp32-level accuracy at 3 MMs/bank
        # of 1 cyc/col instead of fp32's 4 cyc/col.
        f32r = mybir.dt.float32r
        latHiT = nc.dram_tensor("latHiT", [D, rows], f32r,
                                kind="ExternalInput").ap()
        latLoT = nc.dram_tensor("latLoT", [D, rows], f32r,
                                kind="ExternalInput").ap()
        ctHiT = nc.dram_tensor("ctHiT", [D, C], f32r,
                               kind="ExternalInput").ap()
        ctLoT = nc.dram_tensor("ctLoT", [D, C], f32r,
                               kind="ExternalInput").ap()
    if variant == "rank1":
        ones1 = nc.dram_tensor("ones1", [1, P], F32, kind="ExternalInput").ap()
        hneg1 = nc.dram_tensor("hneg1", [1, C], F32, kind="ExternalInput").ap()
    out_dt = mybir.dt.uint32 if variant == "max8" else F32
    out = nc.dram_tensor("out", [P, n_tiles], out_dt, kind="ExternalOutput").ap()

    # Load the latent shard in independent chunks so compute starts after
    # chunk 0 instead of after the whole 8MB (single-queue SWDGE would
    # serialize ~300us of load ahead of the first matmul).
    n_chunks = max(1, min(16, n_tiles))
    while n_tiles % n_chunks:
        n_chunks -= 1
    tpc = n_tiles // n_chunks

    lat_streams = 2 if variant.startswith("limb") else 1
    with tile.TileContext(nc) as tc:
        with (
            tc.tile_pool(name="const", bufs=1) as cpool,
            tc.tile_pool(name="lat", bufs=n_chunks * lat_streams) as lpool,
            tc.tile_pool(name="scan", bufs=2 if variant.startswith("limb") else 3) as mpool,
            tc.tile_pool(name="scr", bufs=1 if variant.startswith("limb") else 2) as spool,
            tc.tile_pool(name="oacc", bufs=1) as opool,
            tc.tile_pool(name="ps", bufs=4 if variant == "limb4" else 2, space="PSUM") as pspool,
        ):
            ct = cpool.tile([P, C], mm_dt)
            nc.gpsimd.dma_start(out=ct[:], in_=coordsT)
            ht = cpool.tile([P, C], F32)
            nc.gpsimd.dma_start(out=ht[:], in_=hb)
            oacc = opool.tile([P, n_tiles], out_dt)

            if variant == "rank1":
                onest = cpool.tile([1, P], F32)
                nc.gpsimd.dma_start(out=onest[:], in_=ones1)
                hnt = cpool.tile([1, C], F32)
                nc.gpsimd.dma_start(out=hnt[:], in_=hneg1)

            # Self-loading fp32 matmuls (LDW struct) accept only ONE sync
            # wait. Re-write every matmul-read tile in place on ACT so all
            # matmul input deps collapse onto the single ACT proc tick.
            nc.scalar.copy(ct[:], ct[:])
            if variant == "rank1":
                nc.scalar.copy(onest[:], onest[:])
                nc.scalar.copy(hnt[:], hnt[:])

            # whole latent shard stays SBUF-resident (64KB/partition per
            # stream), one tile per chunk so Tile tracks deps independently.
            lat_chunks, lo_chunks = [], []
            if variant.startswith("limb"):
                cth = cpool.tile([P, C], mybir.dt.float32r)
                nc.gpsimd.dma_start(out=cth[:], in_=ctHiT)
                ctl = cpool.tile([P, C], mybir.dt.float32r)
                nc.gpsimd.dma_start(out=ctl[:], in_=ctLoT)
                nc.scalar.copy(cth[:], cth[:])
                nc.scalar.copy(ctl[:], ctl[:])
                for ci in range(n_chunks):
                    sl = slice(ci * tpc * P, (ci + 1) * tpc * P)
                    lh = lpool.tile([P, tpc * P], mybir.dt.float32r, tag="latc")
                    nc.sync.dma_start(out=lh[:], in_=latHiT[:, sl])
                    nc.scalar.copy(lh[:], lh[:])
                    lat_chunks.append(lh)
                    ll = lpool.tile([P, tpc * P], mybir.dt.float32r, tag="latc")
                    nc.sync.dma_start(out=ll[:], in_=latLoT[:, sl])
                    nc.scalar.copy(ll[:], ll[:])
                    lo_chunks.append(ll)
            else:
                for ci in range(n_chunks):
                    lc = lpool.tile([P, tpc * P], mm_dt, tag="latc")
                    nc.sync.dma_start(
                        out=lc[:], in_=latT[:, ci * tpc * P:(ci + 1) * tpc * P])
                    nc.scalar.copy(lc[:], lc[:])
                    lat_chunks.append(lc)

            # repeats>1 re-runs the whole tile pipeline in one NEFF (same
            # outputs overwritten) — used only to measure steady-state
            # kernel time as a slope, independent of dispatch overhead.
            for t in [t for _ in range(repeats) for t in range(n_tiles)]:
                tsl = slice((t % tpc) * P, (t % tpc + 1) * P)
                lat = lat_chunks[t // tpc][:, tsl]
                if variant == "limb4":
                    # half-tile PSUM buffers (4x[P,1024]) + chained scans:
                    # finer PE/DVE pipelining than 2 full-tile buffers.
                    lo = lo_chunks[t // tpc][:, tsl]
                    m = mpool.tile([P, C], F32)
                    halves = []
                    for hh in range(2):
                        ph = pspool.tile([P, C // 2], F32, tag="psh")
                        for k in range(2):
                            gl = slice((2 * hh + k) * MM_N,
                                       (2 * hh + k + 1) * MM_N)
                            psl = slice(k * MM_N, (k + 1) * MM_N)
                            nc.tensor.matmul(ph[:, psl], lhsT=lat,
                                             rhs=cth[:, gl],
                                             start=True, stop=False)
                            nc.tensor.matmul(ph[:, psl], lhsT=lat,
                                             rhs=ctl[:, gl],
                                             start=False, stop=False)
                            nc.tensor.matmul(ph[:, psl], lhsT=lo,
                                             rhs=ct[:, gl],
                                             start=False, stop=True)
                        halves.append(ph)
                    nc.vector._custom_dve(
                        SCANMAX, out=m[:, 0:C // 2],
                        in0=halves[0][:], in1=ht[:, 0:C // 2])
                    nc.vector._custom_dve(
                        SCANMAX_INIT, out=m[:, C // 2:C],
                        in0=halves[1][:], in1=ht[:, C // 2:C],
                        s0=m[:, C // 2 - 1:C // 2])
                    vstar = m[:, C - 1:C]
                    sgn = spool.tile([P, C], F32)
                    nc.scalar.activation(
                        out=sgn[:], in_=m[:],
                        func=mybir.ActivationFunctionType.Sign,
                        bias=vstar, scale=-1.0,
                        accum_out=oacc[:, t:t + 1],
                    )
                    continue
                ps = pspool.tile([P, C], F32)
                for k in range(C // MM_N):
                    sl = slice(k * MM_N, (k + 1) * MM_N)
                    if variant == "limb":
                        lo = lo_chunks[t // tpc][:, tsl]
                        nc.tensor.matmul(ps[:, sl], lhsT=lat, rhs=cth[:, sl],
                                         start=True, stop=False)
                        nc.tensor.matmul(ps[:, sl], lhsT=lat, rhs=ctl[:, sl],
                                         start=False, stop=False)
                        nc.tensor.matmul(ps[:, sl], lhsT=lo, rhs=ct[:, sl],
                                         start=False, stop=True)
                        continue
                    nc.tensor.matmul(
                        ps[:, sl], lhsT=lat, rhs=ct[:, sl],
                        start=True, stop=(variant != "rank1"),
                    )
                    if variant == "rank1":
                        # accumulate ones^T @ (-h) so PSUM holds cross - h
                        nc.tensor.matmul(
                            ps[:, sl], lhsT=onest[:], rhs=hnt[:, sl],
                            start=False, stop=True,
                        )

                if variant == "max8":
                    v = mpool.tile([P, C], F32, tag="vtmp")
                    nc.vector.tensor_sub(v[:], ps[:], ht[:])
                    mx = spool.tile([P, 8], F32, tag="mx8")
                    nc.vector.max(out=mx[:], in_=v[:])
                    ix = spool.tile([P, 8], mybir.dt.uint32, tag="ix8")
                    nc.vector.max_index(ix[:], mx[:], v[:])
                    nc.vector.tensor_copy(oacc[:, t:t + 1], ix[:, 0:1])
                    continue

                m = mpool.tile([P, C], F32)
                if variant in ("fused", "limb"):
                    nc.vector._custom_dve(SCANMAX, out=m[:], in0=ps[:], in1=ht[:])
                elif variant == "rank1":
                    nc.vector.tensor_tensor_scan(
                        out=m[:], data0=ps[:], data1=ht[:],
                        initial=-3.0e38,
                        op0=mybir.AluOpType.max, op1=mybir.AluOpType.bypass,
                    )
                else:
                    v = mpool.tile([P, C], F32, tag="vtmp")
                    nc.vector.tensor_sub(v[:], ps[:], ht[:])
                    nc.vector.tensor_tensor_scan(
                        out=m[:], data0=v[:], data1=v[:],
                        initial=-3.0e38,
                        op0=mybir.AluOpType.max, op1=mybir.AluOpType.bypass,
                    )

                vstar = m[:, C - 1:C]
                if count_engine == "act":
                    sgn = spool.tile([P, C], F32)
                    nc.scalar.activation(
                        out=sgn[:], in_=m[:],
                        func=mybir.ActivationFunctionType.Sign,
                        bias=vstar, scale=-1.0,
                        accum_out=oacc[:, t:t + 1],
                    )
                else:  # 'dve' fallback: exact ALU compare + accum on DVE
                    lt = spool.tile([P, C], F32)
                    nc.vector.scalar_tensor_tensor(
                        out=lt[:], in0=m[:], scalar=vstar, in1=m[:],
                        op0=mybir.AluOpType.is_lt,
                        op1=mybir.AluOpType.bypass,
                        accum_out=oacc[:, t:t + 1],
                    )

            nc.gpsimd.dma_start(out=out, in_=oacc[:])

    _strip_pe_self_waits(nc)
    # Bacc defers reg-alloc / wait-splitting to its compile pipeline, which
    # runs in finalize(); the bass2jax/axon exec path does not call it.
    nc.finalize()
    return nc


def _strip_pe_self_waits(nc):
    """Self-loading fp32 matmuls lower to an LDW struct that accepts only ONE
    sync wait. Tile emits a redundant same-engine (PE-sem) wait for PSUM-slot
    WAW reuse on top of the cross-engine reader-release wait; MM execution is
    strict-FIFO on PE (and LDW never touches PSUM/SBUF-writes), so the
    same-engine wait is timing-irrelevant. Drop PE-updated sems from matmul
    waits when more than one wait is present."""
    pe_sems = set()
    for blk in nc.m.functions[0].blocks:
        for i in blk.instructions:
            if getattr(i, "engine", None) == mybir.EngineType.PE and i.sync_info:
                for u in i.sync_info.on_update:
                    pe_sems.add(u.ant_name)
    for blk in nc.m.functions[0].blocks:
        for i in blk.instructions:
            if type(i).__name__ not in ("InstMatmult", "InstLdweights"):
                continue
            si = i.sync_info
            if not si or len(si.on_wait) <= 1:
                continue
            kept = [w for w in si.on_wait if w.ant_name not in pe_sems]
            if len(kept) != len(si.on_wait):
                assert kept, f"{i.name}: all waits were PE-self waits"
                si.on_wait = kept
                i.sync_info = si


# ---------------------------------------------------------------- host side
def _prep_core_inputs(latent: np.ndarray, coords: np.ndarray,
                      variant: str = "fused"):
    coords = np.asarray(coords, dtype=np.float32)
    latent = np.asarray(latent, dtype=np.float32)
    c2 = np.sum(coords * coords, axis=1, dtype=np.float32)
    h = (0.5 * c2).astype(np.float32)
    coordsT = np.ascontiguousarray(coords.T)                    # [128, 2048]
    hb = np.ascontiguousarray(np.broadcast_to(h[None, :], (P, C)))
    limb = variant.replace("_f32r", "").startswith("limb")
    if limb:
        import ml_dtypes
        c_hi = coords.astype(ml_dtypes.bfloat16).astype(np.float32)
        c_lo = coords - c_hi
        l_hi = latent.astype(ml_dtypes.bfloat16).astype(np.float32)
        l_lo = latent - l_hi
    in_maps = []
    for i in range(N_CORES):
        sl = slice(i * SHARD, (i + 1) * SHARD)
        m = {"coordsT": coordsT, "hb": hb}
        if limb:
            m["latHiT"] = np.ascontiguousarray(l_hi[sl].T)
            m["latLoT"] = np.ascontiguousarray(l_lo[sl].T)
            m["ctHiT"] = np.ascontiguousarray(c_hi.T)
            m["ctLoT"] = np.ascontiguousarray(c_lo.T)
        else:
            m["latT"] = np.ascontiguousarray(latent[sl].T)      # [128, 16384]
        if variant == "rank1":
            m["ones1"] = np.ones((1, P), np.float32)
            m["hneg1"] = np.ascontiguousarray(-h[None, :])
        in_maps.append(m)
    return in_maps


_NC_CACHE: dict = {}


def _get_nc(variant: str, count_engine: str):
    key = (variant, count_engine, SHARD // P)
    if key not in _NC_CACHE:
        _NC_CACHE[key] = build_nc(SHARD // P, variant, count_engine)
    return _NC_CACHE[key]


def run_on_cores(latent, coords, variant="fused", count_engine="act",
                 trace=False):
    from concourse.bass_utils import run_bass_kernel_spmd

    nc = _get_nc(variant, count_engine)
    in_maps = _prep_core_inputs(latent, coords, variant)
    res = run_bass_kernel_spmd(nc, in_maps, core_ids=list(range(N_CORES)),
                               trace=trace)
    shards = []
    for i in range(N_CORES):
        o = res.results[i]["out"]                    # [128, n_tiles]
        o = np.rint(o) if o.dtype == np.float32 else o
        shards.append(o.astype(np.int32).T.reshape(-1))
    return np.concatenate(shards), res


# "limb": f32r limb-split matmul (cross = xh.ch + xh.cl + xl.c), 3 MMs/bank
# at 1 cyc/col vs fp32's 4 -> predicted makespan 369us vs 453us, HW-measured
# 2/131072 tie-level mismatches (rel err 5.9e-06) = fp32-class accuracy.
# "fused" (plain fp32 matmul, 453us, 1/131072) kept as conservative fallback;
# "fused_f32r" (311us) trades accuracy (47 flips, 6.7e-4).
DEFAULT_VARIANT = "limb"


def kernel(latent: np.ndarray, coords: np.ndarray) -> np.ndarray:
    idx, _ = run_on_cores(latent, coords, variant=DEFAULT_VARIANT)
    return idx
